# revision 20
# baseline (speedup 1.0000x reference)
"""Trainium2 Bass kernel for CAAN cross-asset attention.

Reference computation (per batch b of 8):
    q = x @ Wq + bq ; k = x @ Wk + bk ; v = x @ Wv + bv
    beta = softmax((q @ k^T) / 16, axis=-1)          # (N, N), N=2048
    out  = (beta @ v) @ Ww + bw                      # (N,)

Algebraic folding (exact up to fp error):
    A   = (Wq @ Wk^T) / 16          (host fold, f64->f32)
    b[m]   = x_m.(Wk bq) / 16       (host fold per batch)
    u0[m]  = x_m.(Wv Ww)            (host fold per batch)
    t[n,m] = x_n A x_m^T            (device; row-constant terms cancel in softmax)
    p[m,n] = exp(t[n,m])            (device, NO bias: e^{b_m} folded into weights)
    numer[n] = sum_m (u0[m] e^{b_m}) p[m,n]
    denom[n] = sum_m (e^{b_m}) p[m,n]
    out[n] = numer/denom + (bw + bv.Ww)

Device kernel (SPMD, 1 batch element per NeuronCore, 8 cores):
    - xT loaded bf16 [128, 2, 2048]; GT = A-projection (TensorE, f32 PSUM)
    - per key-chunk j (16 chunks of 128 keys):
        t_psum = xT_j^T @ GT  (2x [128,1024] f32 PSUM tiles)
        p_j    = exp(t_psum)  (ScalarE, no bias/scale, bf16 out)
        nd     += ndw_j^T @ p_j  (4 col-group-packed M=2 matmuls, concurrent)
      nd matmuls for j are issued AFTER scores for j+1 (software pipeline)
      so the PE never stalls waiting on the ACT.
    - DMA packed [numer; denom] out; final divide + bw_eff on host
"""

import numpy as np
import ml_dtypes
from contextlib import ExitStack

import concourse.bass as bass
import concourse.tile as tile
from concourse import bacc, mybir
from concourse.bass_utils import run_bass_kernel_spmd

N = 2048          # assets per batch element
H = 256           # hidden
NCORES = 8
P = 128           # partitions
HC = H // P       # h chunks (2)
MC = N // P       # m (key) chunks (16)

F32 = mybir.dt.float32
BF16 = mybir.dt.bfloat16
EXP = mybir.ActivationFunctionType.Exp


def _kernel_body(ctx: ExitStack, tc: "tile.TileContext", out_ap, x_ap, a_ap, ndw_ap):
    nc = tc.nc

    sbuf = ctx.enter_context(tc.tile_pool(name="sbuf", bufs=1))
    psum = ctx.enter_context(tc.tile_pool(name="psum", bufs=3, space="PSUM"))

    # xT[p, hc, n] = x[n, hc*128+p]; host supplies x as 4 chunks [q][p][hc][512]
    # so each DMA is one 256KB transfer with 2KB-contiguous per-partition runs.
    xT = sbuf.tile([P, HC, N], BF16)
    a_sb = sbuf.tile([P, HC, H], BF16)
    ndw = sbuf.tile([P, MC, 2], BF16)
    x_r = x_ap.rearrange("(q p) (c n) -> q p c n", p=P, c=HC)
    nc.sync.dma_start(out=xT[:, :, 0:512], in_=x_r[0])
    # A weights (pre-scaled by 1/16 on host): a_sb[p, ic, o] = A[ic*128+p, o]
    nc.sync.dma_start(out=a_sb, in_=a_ap.rearrange("p (c o) -> p c o", c=HC))
    # nd weights [u0*e^b | e^b]: ndw[p, j, c] = w[j*128+p, c]
    nc.scalar.dma_start(out=ndw, in_=ndw_ap.rearrange("p (j c) -> p j c", j=MC))
    nc.scalar.dma_start(out=xT[:, :, 512:1024], in_=x_r[1])
    nc.sync.dma_start(out=xT[:, :, 1024:1536], in_=x_r[2])
    nc.scalar.dma_start(out=xT[:, :, 1536:2048], in_=x_r[3])

    # warm the PE's HAM clock gate with dummy matmuls while the DMA streams;
    # the real GT matmuls then start at 2.4 GHz instead of 1.2
    warm = sbuf.tile([P, 256], BF16)
    nc.vector.memset(warm, 0.0)
    wps = psum.tile([P, 128], F32, tag="nd", bufs=1)
    for _ in range(52):
        nc.tensor.matmul(wps, warm[:, 0:128], warm[:, 128:256], start=True, stop=True)

    # GT[p, oc, n] = (x@A)[n, oc*128+p]
    GT = sbuf.tile([P, HC, N], BF16)
    for nq in range(4):
        g = psum.tile([P, 1024], F32, name=f"g{nq}", tag="s")
        for oc in range(HC):
            for ic in range(HC):
                nc.tensor.matmul(
                    g[:, oc * 512:(oc + 1) * 512],
                    a_sb[:, ic, oc * 128:(oc + 1) * 128],
                    xT[:, ic, nq * 512:(nq + 1) * 512],
                    start=(ic == 0),
                    stop=(ic == HC - 1),
                )
        nc.vector.tensor_copy(GT[:, 0, nq * 512:(nq + 1) * 512], g[:, 0:512])
        nc.vector.tensor_copy(GT[:, 1, nq * 512:(nq + 1) * 512], g[:, 512:1024])

    # [numer; denom] packed in ONE PSUM bank: n-block s4 (512 wide) sits at
    # partitions 32*s4 + {0 (numer), 1 (denom)} via col-group tile_position.
    nd4 = psum.tile([P, 512], F32, tag="nd", bufs=1)
    nc.vector.memset(nd4, 0.0)

    # ---- main attention loop, software-pipelined ----
    p_tiles = [None] * MC

    def nd_pack(j):
        # 4 back-to-back M=2 matmuls at 4 distinct col groups -> concurrent PE
        for s4 in range(4):
            base = 32 * s4
            nc.tensor.matmul(
                nd4[base:base + 2, :],
                ndw[:, j, :],
                p_tiles[j][:, s4 * 512:(s4 + 1) * 512],
                start=(j == 0),
                stop=(j == MC - 1),
                tile_position=(0, base),
            )

    for j in range(MC):
        ts = [psum.tile([P, 1024], F32, name=f"t{j}h{h}", tag="s") for h in range(2)]
        for ic in range(HC):
            for h in range(2):
                for s in range(2):
                    nc.tensor.matmul(
                        ts[h][:, s * 512:(s + 1) * 512],
                        xT[:, ic, j * 128:(j + 1) * 128],
                        GT[:, ic, h * 1024 + s * 512: h * 1024 + (s + 1) * 512],
                        start=(ic == 0),
                        stop=(ic == HC - 1),
                    )
        if j >= 1:
            nd_pack(j - 1)
        pj = sbuf.tile([P, N], BF16, name=f"p{j}", tag="p", bufs=3)
        p_tiles[j] = pj
        nc.scalar.activation(pj[:, 0:1024], ts[0], EXP)
        nc.scalar.activation(pj[:, 1024:2048], ts[1], EXP)
    nd_pack(MC - 1)

    # evacuate packed [numer; denom] (host divides + reassembles)
    ob = sbuf.tile([P, 512], F32)
    nc.vector.tensor_copy(ob, nd4)
    nc.sync.dma_start(out_ap, ob)


def build_program():
    nc = bacc.Bacc("TRN2", target_bir_lowering=False, debug=False)
    x_ap = nc.dram_tensor("x", [4 * P, HC * 512], BF16, kind="ExternalInput").ap()
    a_ap = nc.dram_tensor("wa", [P, HC * H], BF16, kind="ExternalInput").ap()
    ndw_ap = nc.dram_tensor("ndw", [P, MC * 2], BF16, kind="ExternalInput").ap()
    out_ap = nc.dram_tensor("out", [P, 512], F32, kind="ExternalOutput").ap()
    with tile.TileContext(nc) as tc:
        with ExitStack() as ctx:
            _kernel_body(ctx, tc, out_ap, x_ap, a_ap, ndw_ap)
    nc.compile()
    return nc


_PROGRAM = None


def _get_program():
    global _PROGRAM
    if _PROGRAM is None:
        _PROGRAM = build_program()
    return _PROGRAM


def host_fold(x, Wq, bq, Wk, bk, Wv, bv, Ww, bw):
    """Fold projection weights and per-key bias terms (f64 accumulate)."""
    A16 = (Wq.astype(np.float64) @ Wk.astype(np.float64).T / 16.0).astype(np.float32)
    kb = Wk.astype(np.float64) @ bq.astype(np.float64)          # (H,)
    vw = Wv.astype(np.float64) @ Ww.astype(np.float64)[:, 0]    # (H,)
    xf = x.astype(np.float64)
    b = (xf @ kb) / 16.0                                        # (B, N)
    u0 = xf @ vw                                                # (B, N)
    eb = np.exp(b)
    ndw = np.stack([u0 * eb, eb], axis=-1).astype(np.float32)   # (B, N, 2)
    bw_eff = np.float32(bw[0] + bv.astype(np.float64) @ Ww.astype(np.float64)[:, 0])
    return A16, ndw, bw_eff


def run(x, Wq, bq, Wk, bk, Wv, bv, Ww, bw, trace=False):
    """Returns (out [8, N], BassKernelResults)."""
    x = np.asarray(x, dtype=np.float32)
    A16, ndw, bw_eff = host_fold(
        x, np.asarray(Wq), np.asarray(bq), np.asarray(Wk), np.asarray(bk),
        np.asarray(Wv), np.asarray(bv), np.asarray(Ww), np.asarray(bw),
    )
    # device layout: x16[b, q, p, c, n'] = x[b, q*512+n', c*128+p] (4 chunks of
    # 512 asset-columns, each contiguous per partition)
    x16 = np.ascontiguousarray(
        x.astype(ml_dtypes.bfloat16).transpose(0, 2, 1)     # [B, H, N]
        .reshape(NCORES, HC, P, 4, 512).transpose(0, 3, 2, 1, 4)  # [B, 4, P, HC, 512]
        .reshape(NCORES, 4 * P, HC * 512)
    )
    # A16b[p, c*H+o] = A16[c*128+p, o]
    A16b = np.ascontiguousarray(
        A16.astype(ml_dtypes.bfloat16).reshape(HC, P, H).transpose(1, 0, 2).reshape(P, HC * H)
    )
    # ndw16[b, p, j*2+c] = ndw[b, j*128+p, c]
    ndw16 = np.ascontiguousarray(
        ndw.astype(ml_dtypes.bfloat16).reshape(NCORES, MC, P, 2).transpose(0, 2, 1, 3).reshape(NCORES, P, MC * 2)
    )

    nc = _get_program()
    in_maps = [
        {"x": x16[b], "wa": A16b, "ndw": ndw16[b]}
        for b in range(NCORES)
    ]
    last_err = None
    for attempt in range(3):
        try:
            res = run_bass_kernel_spmd(nc, in_maps, list(range(NCORES)), trace=trace)
            break
        except Exception as e:  # transient NRT device wedges have been observed
            last_err = e
            if attempt == 2:
                raise
            import time as _time
            _time.sleep(20 * (attempt + 1))

    def _final(o):
        numer = np.concatenate([o[0], o[32], o[64], o[96]])
        denom = np.concatenate([o[1], o[33], o[65], o[97]])
        return numer / denom + bw_eff

    out = np.stack([_final(res.results[b]["out"]) for b in range(NCORES)], axis=0)
    return out.astype(np.float32), res


def kernel(x, Wq, bq, Wk, bk, Wv, bv, Ww, bw):
    out, _ = run(x, Wq, bq, Wk, bk, Wv, bv, Ww, bw)
    return out


if __name__ == "__main__":
    rng = np.random.default_rng(0)
    s = 1.0 / np.sqrt(H)
    inputs = {
        "x": rng.standard_normal((8, N, H), dtype=np.float32),
        "Wq": rng.uniform(-s, s, (H, H)).astype(np.float32),
        "bq": rng.uniform(-s, s, (H,)).astype(np.float32),
        "Wk": rng.uniform(-s, s, (H, H)).astype(np.float32),
        "bk": rng.uniform(-s, s, (H,)).astype(np.float32),
        "Wv": rng.uniform(-s, s, (H, H)).astype(np.float32),
        "bv": rng.uniform(-s, s, (H,)).astype(np.float32),
        "Ww": rng.uniform(-s, s, (H, 1)).astype(np.float32),
        "bw": rng.uniform(-s, s, (1,)).astype(np.float32),
    }
    out = kernel(**inputs)
    print("kernel out:", out.shape, out.dtype, out[0, :4])


# revision 21
# speedup vs baseline: 1.0259x; 1.0259x over previous
"""Trainium2 Bass kernel for CAAN cross-asset attention.

Reference computation (per batch b of 8):
    q = x @ Wq + bq ; k = x @ Wk + bk ; v = x @ Wv + bv
    beta = softmax((q @ k^T) / 16, axis=-1)          # (N, N), N=2048
    out  = (beta @ v) @ Ww + bw                      # (N,)

Algebraic folding (exact up to fp error):
    A   = (Wq @ Wk^T) / 16          (host fold, f64->f32)
    b[m]   = x_m.(Wk bq) / 16       (host fold per batch)
    u0[m]  = x_m.(Wv Ww)            (host fold per batch)
    t[n,m] = x_n A x_m^T            (device; row-constant terms cancel in softmax)
    p[m,n] = exp(t[n,m])            (device, NO bias: e^{b_m} folded into weights)
    numer[n] = sum_m (u0[m] e^{b_m}) p[m,n]
    denom[n] = sum_m (e^{b_m}) p[m,n]
    out[n] = numer/denom + (bw + bv.Ww)

Device kernel (SPMD, 1 batch element per NeuronCore, 8 cores):
    - xT loaded bf16 [128, 2, 2048]; GT = A-projection (TensorE, f32 PSUM)
    - per key-chunk j (16 chunks of 128 keys):
        t_psum = xT_j^T @ GT  (2x [128,1024] f32 PSUM tiles)
        p_j    = exp(t_psum)  (ScalarE, no bias/scale, bf16 out)
        nd     += ndw_j^T @ p_j  (4 col-group-packed M=2 matmuls, concurrent)
      nd matmuls for j are issued AFTER scores for j+1 (software pipeline)
      so the PE never stalls waiting on the ACT.
    - DMA packed [numer; denom] out; final divide + bw_eff on host
"""

import numpy as np
import ml_dtypes
from contextlib import ExitStack

import concourse.bass as bass
import concourse.tile as tile
from concourse import bacc, mybir
from concourse.bass_utils import run_bass_kernel_spmd

N = 2048          # assets per batch element
H = 256           # hidden
NCORES = 8
P = 128           # partitions
HC = H // P       # h chunks (2)
MC = N // P       # m (key) chunks (16)

F32 = mybir.dt.float32
BF16 = mybir.dt.bfloat16
EXP = mybir.ActivationFunctionType.Exp


def _kernel_body(ctx: ExitStack, tc: "tile.TileContext", out_ap, x_ap, a_ap, ndw_ap):
    nc = tc.nc

    sbuf = ctx.enter_context(tc.tile_pool(name="sbuf", bufs=1))
    psum = ctx.enter_context(tc.tile_pool(name="psum", bufs=3, space="PSUM"))

    # xT[p, hc, n] = x[n, hc*128+p]; host supplies x as 4 chunks [q][p][hc][512]
    # so each DMA is one 256KB transfer with 2KB-contiguous per-partition runs.
    xT = sbuf.tile([P, HC, N], BF16)
    a_sb = sbuf.tile([P, HC, H], BF16)
    ndw = sbuf.tile([P, MC, 2], BF16)
    x_r = x_ap.rearrange("(q p) (c n) -> q p c n", p=P, c=HC)
    nc.sync.dma_start(out=xT[:, :, 0:512], in_=x_r[0])
    # A weights (pre-scaled by 1/16 on host): a_sb[p, ic, o] = A[ic*128+p, o]
    nc.sync.dma_start(out=a_sb, in_=a_ap.rearrange("p (c o) -> p c o", c=HC))
    # nd weights [u0*e^b | e^b]: ndw[p, j, c] = w[j*128+p, c]
    nc.scalar.dma_start(out=ndw, in_=ndw_ap.rearrange("p (j c) -> p j c", j=MC))
    nc.scalar.dma_start(out=xT[:, :, 512:1024], in_=x_r[1])
    nc.sync.dma_start(out=xT[:, :, 1024:1536], in_=x_r[2])
    nc.scalar.dma_start(out=xT[:, :, 1536:2048], in_=x_r[3])

    # warm the PE's HAM clock gate with dummy matmuls while the DMA streams;
    # the real GT matmuls then start at 2.4 GHz instead of 1.2
    warm = sbuf.tile([P, 256], BF16)
    nc.vector.memset(warm, 0.0)
    wps = psum.tile([P, 128], F32, tag="nd", bufs=1)
    for _ in range(30):
        nc.tensor.matmul(wps, warm[:, 0:128], warm[:, 128:256], start=True, stop=True)

    # GT[p, oc, n] = (x@A)[n, oc*128+p]
    GT = sbuf.tile([P, HC, N], BF16)
    for nq in range(4):
        g = psum.tile([P, 1024], F32, name=f"g{nq}", tag="s")
        for oc in range(HC):
            for ic in range(HC):
                nc.tensor.matmul(
                    g[:, oc * 512:(oc + 1) * 512],
                    a_sb[:, ic, oc * 128:(oc + 1) * 128],
                    xT[:, ic, nq * 512:(nq + 1) * 512],
                    start=(ic == 0),
                    stop=(ic == HC - 1),
                )
        # evacuate on both DVE and ScalarE in parallel (both idle here)
        nc.vector.tensor_copy(GT[:, 0, nq * 512:(nq + 1) * 512], g[:, 0:512])
        nc.scalar.copy(GT[:, 1, nq * 512:(nq + 1) * 512], g[:, 512:1024])

    # [numer; denom] packed in ONE PSUM bank: n-block s4 (512 wide) sits at
    # partitions 32*s4 + {0 (numer), 1 (denom)} via col-group tile_position.
    nd4 = psum.tile([P, 512], F32, tag="nd", bufs=1)
    nc.vector.memset(nd4, 0.0)

    # ---- main attention loop, software-pipelined ----
    p_tiles = [None] * MC

    def nd_pack(j):
        # 4 back-to-back M=2 matmuls at 4 distinct col groups -> concurrent PE
        for s4 in range(4):
            base = 32 * s4
            nc.tensor.matmul(
                nd4[base:base + 2, :],
                ndw[:, j, :],
                p_tiles[j][:, s4 * 512:(s4 + 1) * 512],
                start=(j == 0),
                stop=(j == MC - 1),
                tile_position=(0, base),
            )

    for j in range(MC):
        ts = [psum.tile([P, 1024], F32, name=f"t{j}h{h}", tag="s") for h in range(2)]
        for ic in range(HC):
            for h in range(2):
                for s in range(2):
                    nc.tensor.matmul(
                        ts[h][:, s * 512:(s + 1) * 512],
                        xT[:, ic, j * 128:(j + 1) * 128],
                        GT[:, ic, h * 1024 + s * 512: h * 1024 + (s + 1) * 512],
                        start=(ic == 0),
                        stop=(ic == HC - 1),
                    )
        if j >= 1:
            nd_pack(j - 1)
        pj = sbuf.tile([P, N], BF16, name=f"p{j}", tag="p", bufs=3)
        p_tiles[j] = pj
        nc.scalar.activation(pj[:, 0:1024], ts[0], EXP)
        nc.scalar.activation(pj[:, 1024:2048], ts[1], EXP)
    nd_pack(MC - 1)

    # evacuate packed [numer; denom] (host divides + reassembles)
    ob = sbuf.tile([P, 512], F32)
    nc.vector.tensor_copy(ob, nd4)
    nc.sync.dma_start(out_ap, ob)


def build_program():
    nc = bacc.Bacc("TRN2", target_bir_lowering=False, debug=False)
    x_ap = nc.dram_tensor("x", [4 * P, HC * 512], BF16, kind="ExternalInput").ap()
    a_ap = nc.dram_tensor("wa", [P, HC * H], BF16, kind="ExternalInput").ap()
    ndw_ap = nc.dram_tensor("ndw", [P, MC * 2], BF16, kind="ExternalInput").ap()
    out_ap = nc.dram_tensor("out", [P, 512], F32, kind="ExternalOutput").ap()
    with tile.TileContext(nc) as tc:
        with ExitStack() as ctx:
            _kernel_body(ctx, tc, out_ap, x_ap, a_ap, ndw_ap)
    nc.compile()
    return nc


_PROGRAM = None


def _get_program():
    global _PROGRAM
    if _PROGRAM is None:
        _PROGRAM = build_program()
    return _PROGRAM


def host_fold(x, Wq, bq, Wk, bk, Wv, bv, Ww, bw):
    """Fold projection weights and per-key bias terms (f64 accumulate)."""
    A16 = (Wq.astype(np.float64) @ Wk.astype(np.float64).T / 16.0).astype(np.float32)
    kb = Wk.astype(np.float64) @ bq.astype(np.float64)          # (H,)
    vw = Wv.astype(np.float64) @ Ww.astype(np.float64)[:, 0]    # (H,)
    xf = x.astype(np.float64)
    b = (xf @ kb) / 16.0                                        # (B, N)
    u0 = xf @ vw                                                # (B, N)
    eb = np.exp(b)
    ndw = np.stack([u0 * eb, eb], axis=-1).astype(np.float32)   # (B, N, 2)
    bw_eff = np.float32(bw[0] + bv.astype(np.float64) @ Ww.astype(np.float64)[:, 0])
    return A16, ndw, bw_eff


def run(x, Wq, bq, Wk, bk, Wv, bv, Ww, bw, trace=False):
    """Returns (out [8, N], BassKernelResults)."""
    x = np.asarray(x, dtype=np.float32)
    A16, ndw, bw_eff = host_fold(
        x, np.asarray(Wq), np.asarray(bq), np.asarray(Wk), np.asarray(bk),
        np.asarray(Wv), np.asarray(bv), np.asarray(Ww), np.asarray(bw),
    )
    # device layout: x16[b, q, p, c, n'] = x[b, q*512+n', c*128+p] (4 chunks of
    # 512 asset-columns, each contiguous per partition)
    x16 = np.ascontiguousarray(
        x.astype(ml_dtypes.bfloat16).transpose(0, 2, 1)     # [B, H, N]
        .reshape(NCORES, HC, P, 4, 512).transpose(0, 3, 2, 1, 4)  # [B, 4, P, HC, 512]
        .reshape(NCORES, 4 * P, HC * 512)
    )
    # A16b[p, c*H+o] = A16[c*128+p, o]
    A16b = np.ascontiguousarray(
        A16.astype(ml_dtypes.bfloat16).reshape(HC, P, H).transpose(1, 0, 2).reshape(P, HC * H)
    )
    # ndw16[b, p, j*2+c] = ndw[b, j*128+p, c]
    ndw16 = np.ascontiguousarray(
        ndw.astype(ml_dtypes.bfloat16).reshape(NCORES, MC, P, 2).transpose(0, 2, 1, 3).reshape(NCORES, P, MC * 2)
    )

    nc = _get_program()
    in_maps = [
        {"x": x16[b], "wa": A16b, "ndw": ndw16[b]}
        for b in range(NCORES)
    ]
    last_err = None
    for attempt in range(3):
        try:
            res = run_bass_kernel_spmd(nc, in_maps, list(range(NCORES)), trace=trace)
            break
        except Exception as e:  # transient NRT device wedges have been observed
            last_err = e
            if attempt == 2:
                raise
            import time as _time
            _time.sleep(20 * (attempt + 1))

    def _final(o):
        numer = np.concatenate([o[0], o[32], o[64], o[96]])
        denom = np.concatenate([o[1], o[33], o[65], o[97]])
        return numer / denom + bw_eff

    out = np.stack([_final(res.results[b]["out"]) for b in range(NCORES)], axis=0)
    return out.astype(np.float32), res


def kernel(x, Wq, bq, Wk, bk, Wv, bv, Ww, bw):
    out, _ = run(x, Wq, bq, Wk, bk, Wv, bv, Ww, bw)
    return out


if __name__ == "__main__":
    rng = np.random.default_rng(0)
    s = 1.0 / np.sqrt(H)
    inputs = {
        "x": rng.standard_normal((8, N, H), dtype=np.float32),
        "Wq": rng.uniform(-s, s, (H, H)).astype(np.float32),
        "bq": rng.uniform(-s, s, (H,)).astype(np.float32),
        "Wk": rng.uniform(-s, s, (H, H)).astype(np.float32),
        "bk": rng.uniform(-s, s, (H,)).astype(np.float32),
        "Wv": rng.uniform(-s, s, (H, H)).astype(np.float32),
        "bv": rng.uniform(-s, s, (H,)).astype(np.float32),
        "Ww": rng.uniform(-s, s, (H, 1)).astype(np.float32),
        "bw": rng.uniform(-s, s, (1,)).astype(np.float32),
    }
    out = kernel(**inputs)
    print("kernel out:", out.shape, out.dtype, out[0, :4])
